# revision 7
# baseline (speedup 1.0000x reference)
"""Trainium2 Bass kernel for DecomposableAttention (B=2, L=4096, E=1024, H=2048, O=3).

Sharding: 8 cores = 2 groups of 4 (one per batch element). Within a group the
sequence dim L is sharded 4 ways (Ls=1024 rows per core). Cross-core data:
  - AllGather of the f-MLP outputs faT/fbT (fp8, for the attention bmm)
  - ReduceScatter of exp-row/col-sum partials (softmax denominators)
  - ReduceScatter / AllReduce for the tiny aggregate h-MLP.
Both attention orientations exp(fa@fb^T) and exp(fb@fa^T) are computed locally
([all x shard] each) so that beta and alpha contractions are fully local.

Precision (validated vs the reference on CPU, rel err ~1.3e-3 vs 2e-2 gate):
  - f/g MLPs: bf16 weights (host-prepacked into stationary tiles) x bf16
    activations, fp32 PSUM accumulate. Full PE rate (1 row/cycle).
  - attention scores: fp8e4 operands (fa/fb scaled x16) with DoubleRow perf
    mode -> 2x PE rate. exp() stored bf16 (scores reach 57.7; fp8 storage
    would underflow quiet rows, so e stays bf16).
  - beta/alpha contractions: bf16; softmax 1/denominator applied to the
    contraction OUTPUT (valid since the scale depends only on the output
    free index), halving DVE work vs scaling the score rows.
  - h-MLP: fp32 (tiny).
"""

import numpy as np

try:
    import concourse.bass as bass
except ImportError:  # fall back to the staged repo checkout
    import sys
    for p in ("/opt/trn_rl_repo", "/root/.axon_site/_ro/trn_rl_repo"):
        if p not in sys.path:
            sys.path.insert(0, p)
    import concourse.bass as bass
import concourse.mybir as mybir
import concourse.tile as tile
from concourse import bacc
from concourse import bass_utils

import ml_dtypes

F32 = mybir.dt.float32
BF16 = mybir.dt.bfloat16
FP8 = mybir.dt.float8e4
AF = mybir.ActivationFunctionType
ALU = mybir.AluOpType
PM = mybir.MatmulPerfMode
P = 128
CH = 512  # moving free-dim chunk (1 fp32 PSUM bank)
SA = 16.0  # fp8 activation scale for fa/fb


def build_nc(B=2, L=4096, E=1024, H=2048, O=3, n_cores=8, reps=1,
             mock_collectives=False, phases=None):
    """Build the SPMD Bass program (identical on all cores; per-core inputs)."""
    G = n_cores // B          # cores per batch group
    Ls = L // G               # sequence shard per core
    ET, HT, IT = E // P, H // P, L // P       # 128-tiles per dim
    CHN = Ls // CH            # free chunks per token block
    KT1 = 2 * H // P          # agg dim tiles (h layer 1)
    KS1 = KT1 // G            # per-core k-tiles for hW1
    KS2 = (H // P) // G       # per-core k-tiles for hW2
    assert Ls % CH == 0 and E % P == 0 and H % P == 0 and (2 * H) % (G * P) == 0
    assert (H // P) % G == 0

    groups = [list(range(g * G, (g + 1) * G)) for g in range(B)]

    nc = bacc.Bacc("TRN2", target_bir_lowering=False, debug=False,
                   num_devices=n_cores)

    # ---------------- external I/O ----------------
    xaT = nc.dram_tensor("xaT", [E, Ls], BF16, kind="ExternalInput")
    xbT = nc.dram_tensor("xbT", [E, Ls], BF16, kind="ExternalInput")
    x1f = nc.dram_tensor("x1f", [L, E], BF16, kind="ExternalInput")
    x2f = nc.dram_tensor("x2f", [L, E], BF16, kind="ExternalInput")
    w_in = {}
    for nm in ("f", "g"):
        # prepacked stationary tiles: [MT, P, KT, P] bf16
        w_in[nm + "W1"] = nc.dram_tensor(nm + "W1", [HT, P, ET, P], BF16,
                                         kind="ExternalInput")
        w_in[nm + "W2"] = nc.dram_tensor(nm + "W2", [HT, P, HT, P], BF16,
                                         kind="ExternalInput")
        w_in[nm + "W3"] = nc.dram_tensor(nm + "W3", [HT, P, HT, P], BF16,
                                         kind="ExternalInput")
        for i in (1, 2, 3):
            w_in[f"{nm}b{i}"] = nc.dram_tensor(f"{nm}b{i}", [H], F32,
                                               kind="ExternalInput")
    fb3s = nc.dram_tensor("fb3s", [H], F32, kind="ExternalInput")  # 16*fb3
    hW1s = nc.dram_tensor("hW1s", [2 * H // G, H], F32, kind="ExternalInput")
    hW2s = nc.dram_tensor("hW2s", [H // G, H], F32, kind="ExternalInput")
    hW3 = nc.dram_tensor("hW3", [H, O], F32, kind="ExternalInput")
    hb1s = nc.dram_tensor("hb1s", [H // G], F32, kind="ExternalInput")
    hb2 = nc.dram_tensor("hb2", [H], F32, kind="ExternalInput")
    hb3 = nc.dram_tensor("hb3", [O], F32, kind="ExternalInput")
    out = nc.dram_tensor("out", [O, 1], F32, kind="ExternalOutput")

    with tile.TileContext(nc) as tc:
        with (
            tc.tile_pool(name="big", bufs=1) as bigp,
            tc.tile_pool(name="med", bufs=1) as medp,
            tc.tile_pool(name="wst", bufs=2) as wstp,
            tc.tile_pool(name="sst", bufs=2) as sstp,
            tc.tile_pool(name="row", bufs=3) as rowp,
            tc.tile_pool(name="ev", bufs=2) as evp,
            tc.tile_pool(name="sml", bufs=1) as smlp,
            tc.tile_pool(name="ps", bufs=8, space="PSUM") as psp,
            tc.tile_pool(name="dram", bufs=1, space="DRAM") as dramp,
        ):
            on = lambda p: phases is None or p in phases
            for _rep in range(reps):
                def cc(kind, op, replica_groups, tin, tout):
                    if not mock_collectives:
                        nc.gpsimd.collective_compute(kind, op,
                                                     replica_groups=replica_groups,
                                                     ins=[tin.opt()],
                                                     outs=[tout.opt()])
                        return
                    if kind == "AllGather":
                        for s in range(G):
                            nc.sync.dma_start(tout[s], tin[:])
                    elif kind == "ReduceScatter":
                        if len(tin.shape) > 1 and tin.shape[0] == G:
                            nc.sync.dma_start(tout[:], tin[0])
                        else:
                            nc.sync.dma_start(tout[:], tin[:tout.shape[0]])
                    else:  # AllReduce
                        nc.sync.dma_start(tout[:], tin[:])

                def dma_split(dst_ap, src_ap, n):
                    K = dst_ap.shape[1]
                    step = max(1, (K + n - 1) // n)
                    for s in range(0, K, step):
                        e = min(K, s + step)
                        nc.sync.dma_start(dst_ap[:, s:e], src_ap[:, s:e])

                # ---------------- DRAM scratch ----------------
                ag_in = dramp.tile([2, H, Ls], FP8)            # faT_q, fbT_q
                ag_outA = dramp.tile([G, H, Ls], FP8)          # gathered fa
                ag_outB = dramp.tile([G, H, Ls], FP8)          # gathered fb
                tA = dramp.tile([IT, P, Ls], BF16)             # exp(S)  [all i, my j]
                tB = dramp.tile([IT, P, Ls], BF16)             # exp(S^T)[all j, my i]
                rc_in = dramp.tile([G, 2, Ls], F32)            # r/c partial sums
                rc_out = dramp.tile([2, Ls], F32)              # RS result (my shard)
                sp_beta = dramp.tile([ET, P, Ls], BF16)        # betaT spill
                sp_alpha = dramp.tile([ET, P, Ls], BF16)
                vs_in = dramp.tile([2 * H], F32)               # agg partial
                vs_out = dramp.tile([2 * H // G], F32)         # RS: my agg k-slice
                h1_in = dramp.tile([H], F32)
                h1_out = dramp.tile([H // G], F32)
                h2_in = dramp.tile([H], F32)
                h2_out = dramp.tile([H], F32)

                # ---------------- bias tiles ----------------
                btiles = {}
                for nm in ("fb1", "fb2", "gb1", "gb2", "gb3"):
                    t = smlp.tile([P, HT], F32, name=f"bt_{nm}", tag=f"bt_{nm}")
                    nc.sync.dma_start(t[:], w_in[nm].rearrange("(m p) -> p m", p=P))
                    btiles[nm] = t
                fb3s_t = smlp.tile([P, HT], F32, tag="bt_fb3s")
                nc.sync.dma_start(fb3s_t[:], fb3s.rearrange("(m p) -> p m", p=P))
                hb1s_t = smlp.tile([P, KS2], F32, tag="hb1s_t")
                nc.sync.dma_start(hb1s_t[:], hb1s.rearrange("(m p) -> p m", p=P))
                hb2_t = smlp.tile([P, HT], F32, tag="hb2_t")
                nc.sync.dma_start(hb2_t[:], hb2.rearrange("(m p) -> p m", p=P))
                hb3_t = smlp.tile([O, 1], F32, tag="hb3_t")
                nc.sync.dma_start(hb3_t[:], hb3[:, None])

                # ---------------- helpers ----------------
                def mlp3(src_ap, kt_in, W1, W2, W3, b1, b2, consume, pfx):
                    """3-layer MLP (feature-major bf16 activations [P, kt, Ls]),
                    ReLU each layer. src_ap: DRAM AP [P, kt_in, Ls] bf16.
                    consume(m, ch, psum) handles the layer-3 psum output."""
                    in_t = medp.tile([P, ET, Ls], BF16, name=f"{pfx}_in",
                                     tag="inacts")
                    dma_split(in_t[:, :kt_in, :], src_ap, 4)
                    h1 = bigp.tile([P, HT, Ls], BF16, name=f"{pfx}_h1", tag="bigA")
                    for m in range(HT):
                        ws = wstp.tile([P, HT, P], BF16, name=f"{pfx}_w1", tag="wst")
                        nc.sync.dma_start(ws[:, :kt_in, :], W1[m])
                        for ch in range(CHN):
                            ps = psp.tile([P, CH], F32, name=f"{pfx}_ps1", tag="ps")
                            for k in range(kt_in):
                                nc.tensor.matmul(
                                    ps[:], ws[:, k, :],
                                    in_t[:, k, ch * CH:(ch + 1) * CH],
                                    start=(k == 0), stop=(k == kt_in - 1))
                            nc.scalar.activation(
                                h1[:, m, ch * CH:(ch + 1) * CH], ps[:],
                                AF.Relu, bias=b1[:, m:m + 1])
                    h2 = bigp.tile([P, HT, Ls], BF16, name=f"{pfx}_h2", tag="bigB")
                    for m in range(HT):
                        ws = wstp.tile([P, HT, P], BF16, name=f"{pfx}_w2", tag="wst")
                        nc.sync.dma_start(ws[:], W2[m])
                        for ch in range(CHN):
                            ps = psp.tile([P, CH], F32, name=f"{pfx}_ps2", tag="ps")
                            for k in range(HT):
                                nc.tensor.matmul(
                                    ps[:], ws[:, k, :],
                                    h1[:, k, ch * CH:(ch + 1) * CH],
                                    start=(k == 0), stop=(k == HT - 1))
                            nc.scalar.activation(
                                h2[:, m, ch * CH:(ch + 1) * CH], ps[:],
                                AF.Relu, bias=b2[:, m:m + 1])
                    for m in range(HT):
                        ws = wstp.tile([P, HT, P], BF16, name=f"{pfx}_w3", tag="wst")
                        nc.sync.dma_start(ws[:], W3[m])
                        for ch in range(CHN):
                            ps = psp.tile([P, CH], F32, name=f"{pfx}_ps3", tag="ps")
                            for k in range(HT):
                                nc.tensor.matmul(
                                    ps[:], ws[:, k, :],
                                    h2[:, k, ch * CH:(ch + 1) * CH],
                                    start=(k == 0), stop=(k == HT - 1))
                            consume(m, ch, ps)

                # ---------------- phase F: f-MLP on x1 shard and x2 shard --------
                # layer-3 output written as fp8e4 scaled x16 straight into
                # ag_in; each orientation's AllGather is issued as soon as its
                # f stream finishes, so F(xb)+g(xa) hide the collectives.
                for a, src in (((0, xaT), (1, xbT)) if on("F") else ()):
                    def f_consume(m, ch, ps, a=a):
                        ev = evp.tile([P, CH], FP8, name="f_ev", tag="evq")
                        nc.scalar.activation(ev[:], ps[:], AF.Relu, scale=SA,
                                             bias=fb3s_t[:, m:m + 1])
                        nc.sync.dma_start(
                            ag_in[a, m * P:(m + 1) * P, ch * CH:(ch + 1) * CH],
                            ev[:])
                    mlp3(src.rearrange("(k p) t -> p k t", p=P), ET,
                         w_in["fW1"], w_in["fW2"], w_in["fW3"],
                         btiles["fb1"], btiles["fb2"], f_consume, f"F{a}")
                    if on("AG"):
                        cc("AllGather", ALU.bypass, groups, ag_in[a],
                           ag_outA if a == 0 else ag_outB)

                # ---------------- g-MLP stream machinery ----------------
                vsum = smlp.tile([P, HT, 4], F32, tag="vsum")

                def g_stream(s, src_ap, pfx):
                    vred = smlp.tile([P, HT, CHN], F32, name=f"{pfx}_vred",
                                     tag="vred")

                    def g_consume(m, ch, ps):
                        ev = evp.tile([P, CH], F32, name="g_ev", tag="ev")
                        nc.scalar.activation(ev[:], ps[:], AF.Relu,
                                             bias=btiles["gb3"][:, m:m + 1])
                        nc.vector.tensor_reduce(vred[:, m, ch:ch + 1], ev[:],
                                                axis=mybir.AxisListType.X,
                                                op=ALU.add)
                    mlp3(src_ap, ET, w_in["gW1"], w_in["gW2"], w_in["gW3"],
                         btiles["gb1"], btiles["gb2"], g_consume, pfx)
                    nc.vector.tensor_reduce(vsum[:, :, s:s + 1], vred[:],
                                            axis=mybir.AxisListType.X, op=ALU.add)

                # g on x1 shard: overlaps the AllGather
                if on("Gxa"):
                    g_stream(0, xaT.rearrange("(k p) t -> p k t", p=P), "Gxa")

                # ---------------- phase S: attention scores, exp, partials -------
                # S_A: tA = exp(fa_full @ fbT_q)   [all i (part-tiles), my j (free)]
                # S_B: tB = exp(fb_full @ faT_q)   [all j (part-tiles), my i (free)]
                # fp8 DoubleRow matmuls (2 k-rows/cycle); exp scale un-does SA^2.
                raccA = smlp.tile([P, IT, CHN], F32, tag="raccA")
                raccB = smlp.tile([P, IT, CHN], F32, tag="raccB")
                rsA = smlp.tile([P, IT], F32, tag="rsA")   # partial row sums
                rsB = smlp.tile([P, IT], F32, tag="rsB")   # partial col sums
                for a, (tdst, racc, rsum) in (
                        enumerate(((tA, raccA, rsA), (tB, raccB, rsB)))
                        if on("S") else ()):
                    mv = sstp.tile([P, HT, Ls], FP8, name="s_mv", tag="smv",
                                   bufs=1)
                    dma_split(mv[:], ag_in[1 - a].rearrange("(k p) t -> p k t",
                                                            p=P), 4)
                    ag_out = ag_outA if a == 0 else ag_outB
                    for im4 in range(IT // 4):
                        st = sstp.tile([P, HT, 4 * P], FP8, name="s_st", tag="sst")
                        gidx, i0 = im4 // 2, (im4 % 2) * 4 * P
                        nc.sync.dma_start(
                            st[:], ag_out[gidx, :, i0:i0 + 4 * P]
                            .rearrange("(k p) i -> p k i", p=P))
                        for sub in range(4):
                            im = im4 * 4 + sub
                            et = rowp.tile([P, Ls], BF16, name="s_exp", tag="row",
                                           bufs=2)
                            for jc in range(CHN):
                                ps = psp.tile([P, CH], F32, name="s_ps", tag="ps")
                                for kk in range(HT // 2):
                                    nc.tensor.matmul(
                                        ps[:],
                                        st[:, 2 * kk:2 * kk + 2,
                                           sub * P:(sub + 1) * P],
                                        mv[:, 2 * kk:2 * kk + 2,
                                           jc * CH:(jc + 1) * CH],
                                        start=(kk == 0), stop=(kk == HT // 2 - 1),
                                        perf_mode=PM.DoubleRow)
                                nc.scalar.activation(
                                    et[:, jc * CH:(jc + 1) * CH], ps[:], AF.Exp,
                                    scale=1.0 / (SA * SA),
                                    accum_out=racc[:, im, jc:jc + 1])
                            nc.sync.dma_start(tdst[im], et[:])
                    nc.vector.tensor_reduce(rsum[:], racc[:],
                                            axis=mybir.AxisListType.X, op=ALU.add)

                # ---------------- ReduceScatter row/col sums ----------------
                # rc_in[s, 0, :] = r partials for i-shard s; [s, 1, :] = c partials.
                mloc = Ls // P
                for s in (range(G) if on("RC") else ()):
                    nc.sync.dma_start(
                        rc_in[s, 0, :].rearrange("(m p) -> p m", p=P),
                        rsA[:, s * mloc:(s + 1) * mloc])
                    nc.sync.dma_start(
                        rc_in[s, 1, :].rearrange("(m p) -> p m", p=P),
                        rsB[:, s * mloc:(s + 1) * mloc])
                if on("RC"):
                    cc("ReduceScatter", ALU.add, groups, rc_in, rc_out)
                # broadcast + reciprocal -> [P, Ls] scale rows (applied to the
                # beta/alpha contraction OUTPUTS, whose free dim is my i / my j)
                def make_inv(which, nm):
                    dst = smlp.tile([P, Ls], F32, name=nm, tag="rcinv", bufs=2)
                    t1 = rowp.tile([1, Ls], F32, name="rc_row", tag="rcrow",
                                   bufs=1)
                    nc.sync.dma_start(t1[:], rc_out[which][None, :])
                    bc = wstp.tile([P, Ls], F32, name="rc_bc", tag="rcbc", bufs=1)
                    nc.gpsimd.partition_broadcast(bc[:], t1[:])
                    nc.vector.reciprocal(dst[:], bc[:])
                    return dst

                # g on x2 shard: fills PE while RC + beta/alpha DMAs run
                if on("Gxb"):
                    g_stream(1, xbT.rearrange("(k p) t -> p k t", p=P), "Gxb")

                rinv = make_inv(0, "rinv") if on("BA") else None
                cinv = make_inv(1, "cinv") if on("BA") else None

                # ---------------- beta / alpha contractions (bf16) ----------
                # betaT[e, i_my] = (sum_j x2[j, e] * tB[j, i_my]) * rinv[i_my]
                # alphaT[e, j_my] = (sum_i x1[i, e] * tA[i, j_my]) * cinv[j_my]
                for xsrc, tsrc, scl, spill, pfx in ((
                        (x2f, tB, rinv, sp_beta, "bt"),
                        (x1f, tA, cinv, sp_alpha, "al")) if on("BA") else ()):
                    xlo = bigp.tile([P, IT // 2, E], BF16, name=f"{pfx}_xlo",
                                    tag="bigA")
                    dma_split(xlo[:],
                              xsrc[:L // 2].rearrange("(k p) e -> p k e", p=P), 8)
                    xhi = bigp.tile([P, IT // 2, E], BF16, name=f"{pfx}_xhi",
                                    tag="bigB")
                    dma_split(xhi[:],
                              xsrc[L // 2:].rearrange("(k p) e -> p k e", p=P), 8)
                    for ch in range(CHN):
                        pss = [psp.tile([P, CH], F32, name=f"{pfx}_ps{e}",
                                        tag="ps") for e in range(ET)]
                        for jk in range(IT):
                            rt = rowp.tile([P, CH], BF16, name=f"{pfx}_rt",
                                           tag="rt", bufs=8)
                            nc.sync.dma_start(rt[:],
                                              tsrc[jk, :, ch * CH:(ch + 1) * CH])
                            xt = xlo if jk < IT // 2 else xhi
                            jl = jk % (IT // 2)
                            for e in range(ET):
                                nc.tensor.matmul(
                                    pss[e][:], xt[:, jl, e * P:(e + 1) * P],
                                    rt[:],
                                    start=(jk == 0), stop=(jk == IT - 1))
                        for e in range(ET):
                            ev = evp.tile([P, CH], BF16, name=f"{pfx}_ev",
                                          tag="evb")
                            nc.vector.tensor_tensor(
                                ev[:], pss[e][:],
                                scl[:, ch * CH:(ch + 1) * CH], ALU.mult)
                            nc.sync.dma_start(
                                spill[e, :, ch * CH:(ch + 1) * CH], ev[:])

                # ---------------- remaining g-MLP streams ----------------
                if on("Gbt"):
                    g_stream(2, sp_beta.rearrange("m p t -> p m t"), "Gbt")
                if on("Gal"):
                    g_stream(3, sp_alpha.rearrange("m p t -> p m t"), "Gal")

                if phases is not None and "H" not in phases:
                    # liveness sink for phase-subset probe builds: touch the
                    # main products so nothing is dead-code eliminated.
                    snk = smlp.tile([P, 8], F32, tag="snk")
                    if "BA" in phases:
                        nc.sync.dma_start(snk[:, 0:1],
                                          sp_beta[0, :, 0:2].bitcast(F32))
                        nc.sync.dma_start(snk[:, 1:2],
                                          sp_alpha[0, :, 0:2].bitcast(F32))
                    elif "S" in phases:
                        nc.sync.dma_start(snk[:, 0:1], tA[0, :, 0:2].bitcast(F32))
                        nc.sync.dma_start(snk[:, 1:2], tB[0, :, 0:2].bitcast(F32))
                    elif "AG" in phases or "F" in phases:
                        nc.sync.dma_start(
                            snk[:, 0:2],
                            ag_in[0, :P, 0:8].bitcast(F32))
                    if "Gxa" in phases or "Gxb" in phases or "Gbt" in phases \
                            or "Gal" in phases:
                        nc.scalar.copy(snk[:, 2:3], vsum[:, 0, 0:1])
                    ot = smlp.tile([O, 1], F32, tag="ot")
                    nc.vector.tensor_reduce(ot[0:O, 0:1], snk[0:O, :],
                                            axis=mybir.AxisListType.X, op=ALU.add)
                    nc.sync.dma_start(out[:], ot[:])

                if on("H"):
                    # ---------------- aggregate + h-MLP ----------------
                    # v1 = g(x1).sum + g(beta).sum ; v2 = g(x2).sum + g(alpha).sum
                    v12 = smlp.tile([P, HT, 2], F32, tag="v12")
                    nc.vector.tensor_tensor(v12[:, :, 0:1], vsum[:, :, 0:1],
                                            vsum[:, :, 2:3], ALU.add)
                    nc.vector.tensor_tensor(v12[:, :, 1:2], vsum[:, :, 1:2],
                                            vsum[:, :, 3:4], ALU.add)
                    nc.sync.dma_start(vs_in[:H].rearrange("(m p) -> p m", p=P),
                                      v12[:, :, 0])
                    nc.sync.dma_start(vs_in[H:].rearrange("(m p) -> p m", p=P),
                                      v12[:, :, 1])
                    cc("ReduceScatter", ALU.add, groups, vs_in, vs_out)
                    aggT = smlp.tile([P, KS1], F32, tag="aggT")
                    nc.sync.dma_start(aggT[:], vs_out.rearrange("(m p) -> p m", p=P))

                    # h layer 1 (k-split partial -> ReduceScatter -> bias+relu)
                    h1p = smlp.tile([P, HT], F32, tag="h1p")
                    for m in range(HT):
                        ws = wstp.tile([P, KS1, P], F32, name="h1_w", tag="hwst")
                        nc.sync.dma_start(
                            ws[:], hW1s[:, m * P:(m + 1) * P]
                            .rearrange("(k p) m -> p k m", p=P))
                        ps = psp.tile([P, CH], F32, name="h1_ps", tag="ps")
                        for k in range(KS1):
                            nc.tensor.matmul(ps[:, 0:1], ws[:, k, :],
                                             aggT[:, k:k + 1],
                                             start=(k == 0), stop=(k == KS1 - 1))
                        nc.scalar.copy(h1p[:, m:m + 1], ps[:, 0:1])
                    nc.sync.dma_start(h1_in.rearrange("(m p) -> p m", p=P), h1p[:])
                    cc("ReduceScatter", ALU.add, groups, h1_in, h1_out)
                    h1s = smlp.tile([P, KS2], F32, tag="h1s")
                    nc.sync.dma_start(h1s[:], h1_out.rearrange("(m p) -> p m", p=P))
                    nc.vector.tensor_tensor(h1s[:], h1s[:], hb1s_t[:], ALU.add)
                    h1sr = smlp.tile([P, KS2], F32, tag="h1sr")
                    nc.scalar.activation(h1sr[:], h1s[:], AF.Relu)

                    # h layer 2 (k-split partial -> AllReduce -> bias+relu)
                    h2p = smlp.tile([P, HT], F32, tag="h2p")
                    for m in range(HT):
                        ws = wstp.tile([P, KS2, P], F32, name="h2_w", tag="hwst")
                        nc.sync.dma_start(
                            ws[:], hW2s[:, m * P:(m + 1) * P]
                            .rearrange("(k p) m -> p k m", p=P))
                        ps = psp.tile([P, CH], F32, name="h2_ps", tag="ps")
                        for k in range(KS2):
                            nc.tensor.matmul(ps[:, 0:1], ws[:, k, :],
                                             h1sr[:, k:k + 1],
                                             start=(k == 0), stop=(k == KS2 - 1))
                        nc.scalar.copy(h2p[:, m:m + 1], ps[:, 0:1])
                    nc.sync.dma_start(h2_in.rearrange("(m p) -> p m", p=P), h2p[:])
                    cc("AllReduce", ALU.add, groups, h2_in, h2_out)
                    h2s = smlp.tile([P, HT], F32, tag="h2s")
                    nc.sync.dma_start(h2s[:], h2_out.rearrange("(m p) -> p m", p=P))
                    nc.vector.tensor_tensor(h2s[:], h2s[:], hb2_t[:], ALU.add)
                    h2sr = smlp.tile([P, HT], F32, tag="h2sr")
                    nc.scalar.activation(h2sr[:], h2s[:], AF.Relu)

                    # h layer 3 (full, every core; O x 1 output)
                    w3t = smlp.tile([P, HT, O], F32, tag="w3t")
                    nc.sync.dma_start(w3t[:], hW3.rearrange("(k p) o -> p k o", p=P))
                    ps = psp.tile([P, CH], F32, name="h3_ps", tag="ps")
                    for k in range(HT):
                        nc.tensor.matmul(ps[:O, 0:1], w3t[:, k, :], h2sr[:, k:k + 1],
                                         start=(k == 0), stop=(k == HT - 1))
                    ot = smlp.tile([O, 1], F32, tag="ot")
                    nc.scalar.activation(ot[:], ps[:O, 0:1], AF.Relu, bias=hb3_t[:])
                    nc.sync.dma_start(out[:], ot[:])

    nc.compile()
    return nc


def _pack_w(W, P=128):
    """[K, M] f32 -> [MT, P, KT, P] bf16 stationary tiles (ws[p,k,m] = W[k*P+p, mt*P+m])."""
    K, M = W.shape
    return np.ascontiguousarray(
        W.reshape(K // P, P, M // P, P).transpose(2, 1, 0, 3)
    ).astype(ml_dtypes.bfloat16)


def make_in_maps(inputs, B=2, L=4096, E=1024, H=2048, O=3, n_cores=8):
    G = n_cores // B
    Ls = L // G
    BFnp = ml_dtypes.bfloat16
    shared = {}
    for nm in ("fW1", "fW2", "fW3", "gW1", "gW2", "gW3"):
        shared[nm] = _pack_w(np.asarray(inputs[nm], np.float32))
    for nm in ("fb1", "fb2", "fb3", "gb1", "gb2", "gb3", "hW3", "hb2", "hb3"):
        shared[nm] = np.ascontiguousarray(np.asarray(inputs[nm], dtype=np.float32))
    shared["fb3s"] = np.ascontiguousarray(
        16.0 * np.asarray(inputs["fb3"], np.float32))
    hW1 = np.asarray(inputs["hW1"], dtype=np.float32)
    hW2 = np.asarray(inputs["hW2"], dtype=np.float32)
    hb1 = np.asarray(inputs["hb1"], dtype=np.float32)
    x1 = np.asarray(inputs["x1"], dtype=np.float32)
    x2 = np.asarray(inputs["x2"], dtype=np.float32)
    in_maps = []
    for c in range(n_cores):
        g, r = c // G, c % G
        m = dict(shared)
        m["xaT"] = np.ascontiguousarray(
            x1[g, r * Ls:(r + 1) * Ls, :].T).astype(BFnp)
        m["xbT"] = np.ascontiguousarray(
            x2[g, r * Ls:(r + 1) * Ls, :].T).astype(BFnp)
        m["x1f"] = np.ascontiguousarray(x1[g]).astype(BFnp)
        m["x2f"] = np.ascontiguousarray(x2[g]).astype(BFnp)
        k1 = 2 * H // G
        m["hW1s"] = np.ascontiguousarray(hW1[r * k1:(r + 1) * k1, :])
        k2 = H // G
        m["hW2s"] = np.ascontiguousarray(hW2[r * k2:(r + 1) * k2, :])
        m["hb1s"] = np.ascontiguousarray(hb1[r * k2:(r + 1) * k2])
        in_maps.append(m)
    return in_maps


def assemble_out(results, B=2, n_cores=8):
    G = n_cores // B
    return np.stack([results[g * G]["out"][:, 0] for g in range(B)]).astype(
        np.float32)


_NC_CACHE = {}


def kernel(**inputs):
    B, L, E = inputs["x1"].shape
    H = inputs["fW1"].shape[1]
    O = inputs["hW3"].shape[1]
    n_cores = 8
    key = (B, L, E, H, O, n_cores)
    if key not in _NC_CACHE:
        _NC_CACHE[key] = build_nc(B, L, E, H, O, n_cores)
    nc = _NC_CACHE[key]
    in_maps = make_in_maps(inputs, B, L, E, H, O, n_cores)
    res = bass_utils.run_bass_kernel_spmd(nc, in_maps,
                                          core_ids=list(range(n_cores)))
    return assemble_out(res.results, B, n_cores)
